# revision 16
# baseline (speedup 1.0000x reference)
"""Trainium2 Bass kernel for nn_CooccurrenceGraph (label co-occurrence graph attention).

Reference math (B=4096, N=80, H=256):
    q = x @ Wq.T + bq ; k = x @ Wk.T + bk ; v = x @ Wv.T + bv
    scores = (q @ k.T / 16) * cooc[None] * (labels*0.8+0.2)[:,None,:]
    attn = softmax(scores, -1)
    out = (attn @ v) @ Wo.T + bo

Strategy: pure data-parallel over 8 NeuronCores (512 batches each).
Per core, channel-major pipeline; host does all layout prep:
  - x shipped PRE-TRANSPOSED channel-major f16 ("xt" [256, bs*N]) so chunk
    loads are plain contiguous DMAs (no on-device transpose).
  - scores decomposition s^T[m,n] = sum_h z[h,m] x^T[h,n] + w[m], with
    z = A @ x^T + u1 (A = Wq^T Wk, u1 = Wq^T bk folded as per-partition bias
    in the PSUM->SBUF copy), and w = u2.x + c0 computed ON HOST and shipped
    transposed ("wm" [80, 2, bs]: plane 0 = w^T, plane 1 = mask^T).
  - per group of 4 batches: s1 = ps_s + w^T (DVE, bcast over n),
    t2 = s1 * cooc^T/16 (Pool), t3 = t2 * mask^T (Pool), e = Exp(t3) (ACT,
    f16; scores are tiny so no max-subtraction needed).
  - v/Wo folded on host: Wvo = Wo @ Wv; bfin = Wo@bv + bo added to VO rows
    (rows of attn sum to 1 after normalization so +bfin lands on y).
  - VO ones-augmented (col 256 = 1) so attn@VO also yields the softmax
    denominator; per batch only matmul + pure PSUM->SBUF copy; the
    reciprocal + normalize run ONCE per chunk over all 16 batches.
  - y stored f16 as [nchunk, 80, 16, 257] (denominator col shipped as
    garbage); host strips/permutes/casts.
"""

import math
import sys

sys.path.insert(0, "/opt/trn_rl_repo")

import numpy as np

import concourse.bass as bass
import concourse.tile as tile
from concourse import bacc, mybir

B, N, H = 4096, 80, 256
N_CORES = 8
BS = B // N_CORES           # batches per core
GB = 16                     # batches per chunk
TOK = GB * N                # tokens per chunk (1280)
SCALE = 1.0 / math.sqrt(H)

F32 = mybir.dt.float32
F16 = mybir.dt.float16
NP_BF16 = np.float16        # back-compat name (x ships f16 now)

_CACHE = {}


def _bcast(ap2, n, pos):
    """Insert a 0-stride dim of size n into a 2D AP at position pos (1 or 2)."""
    a = ap2.ap
    assert len(a) == 2
    if pos == 1:
        new = [a[0], [0, n], a[1]]
    else:
        new = [a[0], a[1], [0, n]]
    return bass.AP(tensor=ap2.tensor, offset=ap2.offset, ap=new)


def _bcast3(ap3, n):
    """Append a 0-stride dim of size n to a 2D AP (free-dim broadcast)."""
    a = ap3.ap
    assert len(a) == 2
    return bass.AP(tensor=ap3.tensor, offset=ap3.offset, ap=[a[0], a[1], [0, n]])


def build(bs=BS, n_devices=N_CORES, reps=1, hw_loop=False, ablate=None):
    """Build + compile the Bass program for `bs` batches per core.

    reps>1 re-runs the whole body (same I/O) for differential timing.
    hw_loop=True wraps the reps in a hardware For_i loop."""
    key = (bs, n_devices, reps, hw_loop, ablate)
    if key in _CACHE:
        return _CACHE[key]

    assert bs % GB == 0
    nchunk = bs // GB
    ntok = bs * N

    nc = bacc.Bacc("TRN2", target_bir_lowering=False, debug=False,
                   enable_asserts=False, num_devices=n_devices)

    xt_d = nc.dram_tensor("xt", [2 * 128, ntok], F16, kind="ExternalInput").ap()
    wm_d = nc.dram_tensor("wm", [N, 2, bs], F32, kind="ExternalInput").ap()
    aT_d = nc.dram_tensor("aT", [H, H], F16, kind="ExternalInput").ap()
    wvo_d = nc.dram_tensor("wvoT", [H, H], F16, kind="ExternalInput").ap()
    u1_d = nc.dram_tensor("u1", [H], F32, kind="ExternalInput").ap()
    bfin_d = nc.dram_tensor("bfin", [128, H], F32, kind="ExternalInput").ap()
    cooc_d = nc.dram_tensor("coocT", [N, N], F32, kind="ExternalInput").ap()
    y_d = nc.dram_tensor("y", [nchunk, N, GB, H + 1], F16,
                         kind="ExternalOutput").ap()

    with tile.TileContext(nc) as tc:
        with (
            tc.tile_pool(name="const", bufs=1) as constp,
            tc.tile_pool(name="xt", bufs=4) as xtp,
            tc.tile_pool(name="z", bufs=3) as zp,
            tc.tile_pool(name="vo", bufs=3) as vop,
            tc.tile_pool(name="yg", bufs=3) as ygp,
            tc.tile_pool(name="small", bufs=10) as smp,
            tc.tile_pool(name="psA", bufs=2, space="PSUM") as psA,
            tc.tile_pool(name="psS", bufs=2, space="PSUM") as psS,
            tc.tile_pool(name="psV", bufs=2, space="PSUM") as psV,
            tc.tile_pool(name="psY", bufs=2, space="PSUM") as psY,
        ):
            # ---- constants (loaded once) ----
            a_sb = constp.tile([128, 2, H], F16)     # [h_p, h_half, o]
            wvo_sb = constp.tile([128, 2, H], F16)
            nc.sync.dma_start(out=a_sb, in_=aT_d.rearrange("(k p) o -> p k o", p=128))
            nc.sync.dma_start(out=wvo_sb, in_=wvo_d.rearrange("(k p) o -> p k o", p=128))
            u1_sb = constp.tile([128, 2], F32)
            nc.sync.dma_start(out=u1_sb, in_=u1_d.rearrange("(k p) -> p k", p=128))
            cooc_sb = constp.tile([N, N], F32)
            nc.sync.dma_start(out=cooc_sb, in_=cooc_d)
            wm_all = constp.tile([N, 2, bs], F32)
            nc.sync.dma_start(out=wm_all, in_=wm_d)
            if ablate == "pe":
                cz = constp.tile([128, 2, TOK], F16)
                nc.vector.memset(cz, 0.01)
                ce = constp.tile([N, 4, N], F16)
                nc.vector.memset(ce, 0.01)
                cvo = constp.tile([N, GB, H + 1], F16)
                nc.vector.memset(cvo, 0.01)
                cy = constp.tile([N, GB, H + 1], F16)
                nc.vector.memset(cy, 0.0)
            bfin_sb = constp.tile([128, H], F32)
            nc.sync.dma_start(out=bfin_sb, in_=bfin_d)

            def _body():
              for c in range(nchunk):
                t0 = c * TOK
                # ---- X' chunk [h, tok] — plain contiguous loads
                xt = xtp.tile([128, 2, TOK], F16, tag="xt")
                nc.sync.dma_start(
                    out=xt,
                    in_=xt_d.rearrange("(k p) t -> p k t", p=128)[:, :, t0:t0 + TOK],
                )

                # ---- Z = A @ X' + u1, channel-major [o, t], f16
                z_sb = zp.tile([128, 2, TOK], F16, tag="z")
                nq = TOK // 320
                for o in range(2):
                    osl = slice(o * 128, (o + 1) * 128)
                    for hf in range(nq):
                        fsl = slice(hf * 320, (hf + 1) * 320)
                        psq = psA.tile([128, 320], F32, tag="ps_a")
                        nc.tensor.matmul(psq, a_sb[:, 0, osl], xt[:, 0, fsl],
                                         start=True, stop=False)
                        nc.tensor.matmul(psq, a_sb[:, 1, osl], xt[:, 1, fsl],
                                         start=False, stop=True)
                        if hf % 2 == 0:
                            nc.vector.tensor_scalar_add(z_sb[:, o, fsl], psq,
                                                        u1_sb[:, o:o + 1])
                        else:
                            nc.scalar.activation(
                                z_sb[:, o, fsl], psq,
                                mybir.ActivationFunctionType.Identity,
                                bias=u1_sb[:, o:o + 1])

                # ---- VO = x @ Wvo.T + bfin, token-major [m, b, o], ones col
                vo_sb = vop.tile([N, GB, H + 1], F16, tag="vo")
                nc.gpsimd.memset(vo_sb[:, :, H], 1.0)
                for bp in range(GB // 2):
                    psv = psV.tile([N, 2, H], F32, tag="ps_v")
                    for j in range(2):
                        b = bp * 2 + j
                        tsl = slice(b * N, (b + 1) * N)
                        nc.tensor.matmul(psv[:, j, :], xt[:, 0, tsl], wvo_sb[:, 0, :],
                                         start=True, stop=False)
                        nc.tensor.matmul(psv[:, j, :], xt[:, 1, tsl], wvo_sb[:, 1, :],
                                         start=False, stop=True)
                    nc.vector.tensor_add(vo_sb[:, bp * 2:bp * 2 + 2, :H], psv,
                                         _bcast(bfin_sb[:N, :], 2, 1))

                # ---- attention, per group of 4 batches
                y_group = ygp.tile([N, GB, H + 1], F16, tag="yg")
                for g in range(GB // 4):
                    ps_s = psS.tile([N, 4, N], F32, tag="ps_s")
                    for j in range(4):
                        b = g * 4 + j
                        tsl = slice(b * N, (b + 1) * N)
                        nc.tensor.matmul(ps_s[:, j, :], z_sb[:, 0, tsl],
                                         xt[:, 0, tsl], start=True, stop=False)
                        nc.tensor.matmul(ps_s[:, j, :], z_sb[:, 1, tsl],
                                         xt[:, 1, tsl], start=False, stop=True)
                    # s1 = ps_s + w^T (bcast over n), f32
                    s1 = smp.tile([N, 4, N], F32, tag="s1")
                    nc.vector.tensor_add(
                        s1, ps_s, _bcast3(wm_all[:, 0, c * GB + g * 4:c * GB + (g + 1) * 4], N))
                    # t2 = s1 * coocT/16 ; t3 = t2 * mask^T   (Pool, SBUF only)
                    t2 = smp.tile([N, 4, N], F16, tag="t2")
                    nc.gpsimd.tensor_mul(t2, s1, _bcast(cooc_sb, 4, 1))
                    nc.gpsimd.tensor_mul(
                        t2, t2, _bcast3(wm_all[:, 1, c * GB + g * 4:c * GB + (g + 1) * 4], N))
                    e4 = smp.tile([N, 4, N], F16, tag="e4")
                    nc.scalar.activation(e4, t2, mybir.ActivationFunctionType.Exp)
                    for j in range(4):
                        b = g * 4 + j
                        ps_y = psY.tile([N, H + 1], F32, tag="ps_y")
                        nc.tensor.matmul(ps_y, e4[:, j, :], vo_sb[:, b, :],
                                         start=True, stop=True)
                        if j % 2 == 0:
                            nc.vector.tensor_copy(y_group[:, b, :], ps_y)
                        else:
                            nc.scalar.activation(
                                y_group[:, b, :], ps_y,
                                mybir.ActivationFunctionType.Copy)

                # ---- batched normalize: rc = 1/denom col, y *= rc
                rc = smp.tile([N, GB], F16, tag="rc")
                with nc.allow_low_precision(reason="1/denom ~ 1/80, f16 ok"):
                    nc.vector.reciprocal(rc, y_group[:, :, H])
                hb = GB // 2
                nc.vector.tensor_mul(y_group[:, :hb, :H], y_group[:, :hb, :H],
                                     _bcast3(rc[:, :hb], H))
                nc.gpsimd.tensor_mul(y_group[:, hb:, :H], y_group[:, hb:, :H],
                                     _bcast3(rc[:, hb:], H))

                # ---- store chunk output (fully contiguous, denom col included)
                nc.sync.dma_start(out=y_d[c], in_=y_group)

            if hw_loop and reps > 1:
                with tc.For_i(0, reps, 1):
                    _body()
            else:
                for rep in range(reps):
                    _body()

    nc.compile()
    _CACHE[key] = nc
    return nc


def _prep_consts(Wq, bq, Wk, bk, Wv, bv, Wo, bo, cooccurrence):
    Wq = np.asarray(Wq, np.float32)
    Wk = np.asarray(Wk, np.float32)
    Wv = np.asarray(Wv, np.float32)
    Wo = np.asarray(Wo, np.float32)
    bv = np.asarray(bv, np.float32)
    bo = np.asarray(bo, np.float32)
    bq = np.asarray(bq, np.float32)
    bk = np.asarray(bk, np.float32)
    Wvo = Wo @ Wv                                  # vo = x @ Wvo.T
    bfin = Wo @ bv + bo
    A = Wq.T @ Wk                                  # scores = x A x^T + ...
    u1 = Wq.T @ bk
    return {
        "aT": np.ascontiguousarray(A.T).astype(np.float16),
        "wvoT": np.ascontiguousarray(Wvo.T).astype(np.float16),
        "u1": u1.astype(np.float32),
        "bfin": np.ascontiguousarray(np.broadcast_to(bfin, (128, H))).astype(np.float32),
        "coocT": np.ascontiguousarray(np.asarray(cooccurrence, np.float32).T * SCALE),
    }


def shard_inputs(inputs):
    """Full input dict -> list of 8 per-core input maps (kernel tensor names)."""
    x = np.asarray(inputs["x"], np.float32)
    labels = np.asarray(inputs["labels"])
    consts = _prep_consts(inputs["Wq"], inputs["bq"], inputs["Wk"], inputs["bk"],
                          inputs["Wv"], inputs["bv"], inputs["Wo"], inputs["bo"],
                          inputs["cooccurrence"])
    u2 = np.asarray(inputs["Wk"], np.float32).T @ np.asarray(inputs["bq"], np.float32)
    c0 = float(np.asarray(inputs["bq"], np.float32)
               @ np.asarray(inputs["bk"], np.float32))
    xf = x.reshape(B * N, H)
    xT = np.ascontiguousarray(xf.T.astype(np.float16))        # [256, B*N]
    w_all = (xf @ u2 + c0).reshape(B, N)                      # [B, N]
    mask = labels.astype(np.float32) * 0.8 + 0.2              # [B, N]
    in_maps = []
    for i in range(N_CORES):
        t0 = i * BS * N
        wm = np.empty((N, 2, BS), np.float32)
        wm[:, 0, :] = w_all[i * BS:(i + 1) * BS].T
        wm[:, 1, :] = mask[i * BS:(i + 1) * BS].T
        in_maps.append({
            "xt": np.ascontiguousarray(xT[:, t0:t0 + BS * N]),
            "wm": wm,
            **consts,
        })
    return in_maps


def unshard_output(res_list):
    """Per-core y [nchunk, 80, 16, 257] f16 -> full [B, N, H] f32."""
    ys = []
    for r in res_list:
        y = np.asarray(r["y"])[:, :, :, :H].astype(np.float32)  # [nc, N, GB, H]
        ys.append(y.transpose(0, 2, 1, 3).reshape(BS, N, H))    # [bs, N, H]
    return np.concatenate(ys, axis=0)


def kernel(x, Wq, bq, Wk, bk, Wv, bv, Wo, bo, cooccurrence, labels, _trace=False):
    from concourse.bass_utils import run_bass_kernel_spmd
    in_maps = shard_inputs(dict(x=x, Wq=Wq, bq=bq, Wk=Wk, bk=bk, Wv=Wv, bv=bv,
                                Wo=Wo, bo=bo, cooccurrence=cooccurrence,
                                labels=labels))
    nc = build()
    try:
        res = run_bass_kernel_spmd(nc, in_maps, core_ids=list(range(N_CORES)),
                                   trace=_trace)
    except ModuleNotFoundError:
        res = run_bass_kernel_spmd(nc, in_maps, core_ids=list(range(N_CORES)),
                                   trace=False)
    ret = unshard_output(res.results).reshape(B, N, H)
    if _trace:
        kernel._last_results = res
    return ret


# revision 22
# speedup vs baseline: 1.0073x; 1.0073x over previous
"""Trainium2 Bass kernel for nn_CooccurrenceGraph (label co-occurrence graph attention).

Reference math (B=4096, N=80, H=256):
    q = x @ Wq.T + bq ; k = x @ Wk.T + bk ; v = x @ Wv.T + bv
    scores = (q @ k.T / 16) * cooc[None] * (labels*0.8+0.2)[:,None,:]
    attn = softmax(scores, -1)
    out = (attn @ v) @ Wo.T + bo

Strategy: pure data-parallel over 8 NeuronCores (512 batches each).
Per core, channel-major pipeline; host does all layout prep:
  - x shipped PRE-TRANSPOSED channel-major f16 ("xt" [256, bs*N]) so chunk
    loads are plain contiguous DMAs (no on-device transpose).
  - scores decomposition s^T[m,n] = sum_h z[h,m] x^T[h,n] + w[m], with
    z = A @ x^T + u1 (A = Wq^T Wk, u1 = Wq^T bk folded as per-partition bias
    in the PSUM->SBUF copy), and w = u2.x + c0 computed ON HOST and shipped
    transposed ("wm" [80, 2, bs]: plane 0 = w^T, plane 1 = mask^T).
  - per group of 4 batches: s1 = ps_s + w^T (DVE, bcast over n),
    t2 = s1 * cooc^T/16 (Pool), t3 = t2 * mask^T (Pool), e = Exp(t3) (ACT,
    f16; scores are tiny so no max-subtraction needed).
  - v/Wo folded on host: Wvo = Wo @ Wv; bfin = Wo@bv + bo added to VO rows
    (rows of attn sum to 1 after normalization so +bfin lands on y).
  - VO ones-augmented (col 256 = 1) so attn@VO also yields the softmax
    denominator; per batch only matmul + pure PSUM->SBUF copy; the
    reciprocal + normalize run ONCE per chunk over all 16 batches.
  - y stored f16 as [nchunk, 80, 16, 257] (denominator col shipped as
    garbage); host strips/permutes/casts.
"""

import math
import sys

sys.path.insert(0, "/opt/trn_rl_repo")

import numpy as np

import concourse.bass as bass
import concourse.tile as tile
from concourse import bacc, mybir

B, N, H = 4096, 80, 256
N_CORES = 8
BS = B // N_CORES           # batches per core
GB = 16                     # batches per chunk
TOK = GB * N                # tokens per chunk (1280)
SCALE = 1.0 / math.sqrt(H)

F32 = mybir.dt.float32
F16 = mybir.dt.float16
NP_BF16 = np.float16        # back-compat name (x ships f16 now)

_CACHE = {}


def _bcast(ap2, n, pos):
    """Insert a 0-stride dim of size n into a 2D AP at position pos (1 or 2)."""
    a = ap2.ap
    assert len(a) == 2
    if pos == 1:
        new = [a[0], [0, n], a[1]]
    else:
        new = [a[0], a[1], [0, n]]
    return bass.AP(tensor=ap2.tensor, offset=ap2.offset, ap=new)


def _bcast3(ap3, n):
    """Append a 0-stride dim of size n to a 2D AP (free-dim broadcast)."""
    a = ap3.ap
    assert len(a) == 2
    return bass.AP(tensor=ap3.tensor, offset=ap3.offset, ap=[a[0], a[1], [0, n]])


def build(bs=BS, n_devices=N_CORES, reps=1, hw_loop=False, ablate=None):
    """Build + compile the Bass program for `bs` batches per core.

    reps>1 re-runs the whole body (same I/O) for differential timing.
    hw_loop=True wraps the reps in a hardware For_i loop."""
    key = (bs, n_devices, reps, hw_loop, ablate)
    if key in _CACHE:
        return _CACHE[key]

    assert bs % GB == 0
    nchunk = bs // GB
    ntok = bs * N

    nc = bacc.Bacc("TRN2", target_bir_lowering=False, debug=False,
                   enable_asserts=False, num_devices=n_devices)

    xt_d = nc.dram_tensor("xt", [2 * 128, ntok], F16, kind="ExternalInput").ap()
    wm_d = nc.dram_tensor("wm", [N, 2, bs], F32, kind="ExternalInput").ap()
    aT_d = nc.dram_tensor("aT", [H, H], F16, kind="ExternalInput").ap()
    wvo_d = nc.dram_tensor("wvoT", [H, H], F16, kind="ExternalInput").ap()
    u1_d = nc.dram_tensor("u1", [H], F32, kind="ExternalInput").ap()
    bfin_d = nc.dram_tensor("bfin", [128, H], F32, kind="ExternalInput").ap()
    cooc_d = nc.dram_tensor("coocT", [N, N], F32, kind="ExternalInput").ap()
    y_d = nc.dram_tensor("y", [nchunk, N, GB, H + 1], F16,
                         kind="ExternalOutput").ap()

    with tile.TileContext(nc) as tc:
        with (
            tc.tile_pool(name="const", bufs=1) as constp,
            tc.tile_pool(name="xt", bufs=4) as xtp,
            tc.tile_pool(name="z", bufs=3) as zp,
            tc.tile_pool(name="vo", bufs=3) as vop,
            tc.tile_pool(name="yg", bufs=3) as ygp,
            tc.tile_pool(name="small", bufs=10) as smp,
            tc.tile_pool(name="psA", bufs=2, space="PSUM") as psA,
            tc.tile_pool(name="psS", bufs=2, space="PSUM") as psS,
            tc.tile_pool(name="psV", bufs=2, space="PSUM") as psV,
            tc.tile_pool(name="psY", bufs=2, space="PSUM") as psY,
        ):
            # ---- constants (loaded once) ----
            a_sb = constp.tile([128, 2, H], F16)     # [h_p, h_half, o]
            wvo_sb = constp.tile([128, 2, H], F16)
            nc.sync.dma_start(out=a_sb, in_=aT_d.rearrange("(k p) o -> p k o", p=128))
            nc.sync.dma_start(out=wvo_sb, in_=wvo_d.rearrange("(k p) o -> p k o", p=128))
            u1_sb = constp.tile([128, 2], F32)
            nc.sync.dma_start(out=u1_sb, in_=u1_d.rearrange("(k p) -> p k", p=128))
            cooc_sb = constp.tile([N, N], F32)
            nc.sync.dma_start(out=cooc_sb, in_=cooc_d)
            wm_all = constp.tile([N, 2, bs], F32)
            nc.sync.dma_start(out=wm_all, in_=wm_d)
            if ablate == "pe":
                cz = constp.tile([128, 2, TOK], F16)
                nc.vector.memset(cz, 0.01)
                ce = constp.tile([N, 4, N], F16)
                nc.vector.memset(ce, 0.01)
                cvo = constp.tile([N, GB, H + 1], F16)
                nc.vector.memset(cvo, 0.01)
                cy = constp.tile([N, GB, H + 1], F16)
                nc.vector.memset(cy, 0.0)
            bfin_sb = constp.tile([128, H], F32)
            nc.sync.dma_start(out=bfin_sb, in_=bfin_d)

            def _body():
              for c in range(nchunk):
                t0 = c * TOK
                # ---- X' chunk [h, tok] — plain contiguous loads
                xt = xtp.tile([128, 2, TOK], F16, tag="xt")
                nc.sync.dma_start(
                    out=xt,
                    in_=xt_d.rearrange("(k p) t -> p k t", p=128)[:, :, t0:t0 + TOK],
                )

                # ---- Z = A @ X' + u1, channel-major [o, t], f16
                z_sb = zp.tile([128, 2, TOK], F16, tag="z")
                nq = TOK // 320
                for o in range(2):
                    osl = slice(o * 128, (o + 1) * 128)
                    for hf in range(nq):
                        fsl = slice(hf * 320, (hf + 1) * 320)
                        psq = psA.tile([128, 320], F32, tag="ps_a")
                        nc.tensor.matmul(psq, a_sb[:, 0, osl], xt[:, 0, fsl],
                                         start=True, stop=False)
                        nc.tensor.matmul(psq, a_sb[:, 1, osl], xt[:, 1, fsl],
                                         start=False, stop=True)
                        if hf % 2 == 0:
                            nc.vector.tensor_scalar_add(z_sb[:, o, fsl], psq,
                                                        u1_sb[:, o:o + 1])
                        else:
                            nc.scalar.activation(
                                z_sb[:, o, fsl], psq,
                                mybir.ActivationFunctionType.Identity,
                                bias=u1_sb[:, o:o + 1])

                # ---- VO = x @ Wvo.T + bfin, token-major [m, b, o], ones col
                vo_sb = vop.tile([N, GB, H + 1], F16, tag="vo")
                nc.gpsimd.memset(vo_sb[:, :, H], 1.0)
                for bp in range(GB // 2):
                    psv = psV.tile([N, 2, H], F32, tag="ps_v")
                    for j in range(2):
                        b = bp * 2 + j
                        tsl = slice(b * N, (b + 1) * N)
                        nc.tensor.matmul(psv[:, j, :], xt[:, 0, tsl], wvo_sb[:, 0, :],
                                         start=True, stop=False)
                        nc.tensor.matmul(psv[:, j, :], xt[:, 1, tsl], wvo_sb[:, 1, :],
                                         start=False, stop=True)
                    nc.vector.tensor_add(vo_sb[:, bp * 2:bp * 2 + 2, :H], psv,
                                         _bcast(bfin_sb[:N, :], 2, 1))

                # ---- attention, per group of 4 batches
                y_group = ygp.tile([N, GB, H + 1], F16, tag="yg")
                for g in range(GB // 4):
                    ps_s = psS.tile([N, 4, N], F32, tag="ps_s")
                    for j in range(4):
                        b = g * 4 + j
                        tsl = slice(b * N, (b + 1) * N)
                        nc.tensor.matmul(ps_s[:, j, :], z_sb[:, 0, tsl],
                                         xt[:, 0, tsl], start=True, stop=False)
                        nc.tensor.matmul(ps_s[:, j, :], z_sb[:, 1, tsl],
                                         xt[:, 1, tsl], start=False, stop=True)
                    # s1 = ps_s + w^T (bcast over n), f32
                    s1 = smp.tile([N, 4, N], F32, tag="s1")
                    nc.vector.tensor_add(
                        s1, ps_s, _bcast3(wm_all[:, 0, c * GB + g * 4:c * GB + (g + 1) * 4], N))
                    # t2 = s1 * coocT/16 ; t3 = t2 * mask^T   (Pool, SBUF only)
                    t2 = smp.tile([N, 4, N], F16, tag="t2")
                    nc.gpsimd.tensor_mul(t2, s1, _bcast(cooc_sb, 4, 1))
                    nc.gpsimd.tensor_mul(
                        t2, t2, _bcast3(wm_all[:, 1, c * GB + g * 4:c * GB + (g + 1) * 4], N))
                    e4 = smp.tile([N, 4, N], F16, tag="e4")
                    nc.scalar.activation(e4, t2, mybir.ActivationFunctionType.Exp)
                    for j in range(4):
                        b = g * 4 + j
                        ps_y = psY.tile([N, H + 1], F32, tag="ps_y")
                        nc.tensor.matmul(ps_y, e4[:, j, :], vo_sb[:, b, :],
                                         start=True, stop=True)
                        if j % 2 == 0:
                            nc.vector.tensor_copy(y_group[:, b, :], ps_y)
                        else:
                            nc.scalar.activation(
                                y_group[:, b, :], ps_y,
                                mybir.ActivationFunctionType.Copy)

                # ---- batched normalize: rc = 1/denom col, y *= rc
                rc = smp.tile([N, GB], F16, tag="rc")
                with nc.allow_low_precision(reason="1/denom ~ 1/80, f16 ok"):
                    nc.vector.reciprocal(rc, y_group[:, :, H])
                hb = GB // 2
                nc.vector.tensor_mul(y_group[:, :hb, :H], y_group[:, :hb, :H],
                                     _bcast3(rc[:, :hb], H))
                nc.gpsimd.tensor_mul(y_group[:, hb:, :H], y_group[:, hb:, :H],
                                     _bcast3(rc[:, hb:], H))

                # ---- store chunk output (fully contiguous, denom col included)
                nc.sync.dma_start(out=y_d[c], in_=y_group)

            if hw_loop and reps > 1:
                if reps % 2 == 0:
                    with tc.For_i(0, reps // 2, 1):
                        _body()
                        _body()
                else:
                    with tc.For_i(0, reps, 1):
                        _body()
            else:
                for rep in range(reps):
                    _body()

    nc.compile()
    _CACHE[key] = nc
    return nc


def _prep_consts(Wq, bq, Wk, bk, Wv, bv, Wo, bo, cooccurrence):
    Wq = np.asarray(Wq, np.float32)
    Wk = np.asarray(Wk, np.float32)
    Wv = np.asarray(Wv, np.float32)
    Wo = np.asarray(Wo, np.float32)
    bv = np.asarray(bv, np.float32)
    bo = np.asarray(bo, np.float32)
    bq = np.asarray(bq, np.float32)
    bk = np.asarray(bk, np.float32)
    Wvo = Wo @ Wv                                  # vo = x @ Wvo.T
    bfin = Wo @ bv + bo
    A = Wq.T @ Wk                                  # scores = x A x^T + ...
    u1 = Wq.T @ bk
    return {
        "aT": np.ascontiguousarray(A.T).astype(np.float16),
        "wvoT": np.ascontiguousarray(Wvo.T).astype(np.float16),
        "u1": u1.astype(np.float32),
        "bfin": np.ascontiguousarray(np.broadcast_to(bfin, (128, H))).astype(np.float32),
        "coocT": np.ascontiguousarray(np.asarray(cooccurrence, np.float32).T * SCALE),
    }


def shard_inputs(inputs):
    """Full input dict -> list of 8 per-core input maps (kernel tensor names)."""
    x = np.asarray(inputs["x"], np.float32)
    labels = np.asarray(inputs["labels"])
    consts = _prep_consts(inputs["Wq"], inputs["bq"], inputs["Wk"], inputs["bk"],
                          inputs["Wv"], inputs["bv"], inputs["Wo"], inputs["bo"],
                          inputs["cooccurrence"])
    u2 = np.asarray(inputs["Wk"], np.float32).T @ np.asarray(inputs["bq"], np.float32)
    c0 = float(np.asarray(inputs["bq"], np.float32)
               @ np.asarray(inputs["bk"], np.float32))
    xf = x.reshape(B * N, H)
    xT = np.ascontiguousarray(xf.T.astype(np.float16))        # [256, B*N]
    w_all = (xf @ u2 + c0).reshape(B, N)                      # [B, N]
    mask = labels.astype(np.float32) * 0.8 + 0.2              # [B, N]
    in_maps = []
    for i in range(N_CORES):
        t0 = i * BS * N
        wm = np.empty((N, 2, BS), np.float32)
        wm[:, 0, :] = w_all[i * BS:(i + 1) * BS].T
        wm[:, 1, :] = mask[i * BS:(i + 1) * BS].T
        in_maps.append({
            "xt": np.ascontiguousarray(xT[:, t0:t0 + BS * N]),
            "wm": wm,
            **consts,
        })
    return in_maps


def unshard_output(res_list):
    """Per-core y [nchunk, 80, 16, 257] f16 -> full [B, N, H] f32."""
    ys = []
    for r in res_list:
        y = np.asarray(r["y"])[:, :, :, :H].astype(np.float32)  # [nc, N, GB, H]
        ys.append(y.transpose(0, 2, 1, 3).reshape(BS, N, H))    # [bs, N, H]
    return np.concatenate(ys, axis=0)


def kernel(x, Wq, bq, Wk, bk, Wv, bv, Wo, bo, cooccurrence, labels, _trace=False):
    from concourse.bass_utils import run_bass_kernel_spmd
    in_maps = shard_inputs(dict(x=x, Wq=Wq, bq=bq, Wk=Wk, bk=bk, Wv=Wv, bv=bv,
                                Wo=Wo, bo=bo, cooccurrence=cooccurrence,
                                labels=labels))
    nc = build()
    try:
        res = run_bass_kernel_spmd(nc, in_maps, core_ids=list(range(N_CORES)),
                                   trace=_trace)
    except ModuleNotFoundError:
        res = run_bass_kernel_spmd(nc, in_maps, core_ids=list(range(N_CORES)),
                                   trace=False)
    ret = unshard_output(res.results).reshape(B, N, H)
    if _trace:
        kernel._last_results = res
    return ret
